# revision 28
# baseline (speedup 1.0000x reference)
"""Trainium2 Bass kernel v3: masked dot-product attention, one head per core.

Per head: O = softmax(mask ? QK^T/sqrt(d) : -inf) @ V, all in "transposed"
[s, q] layout so PV needs no transpose.

Scores are computed in a log2*128 domain: PSUM y = S*scale*log2e*128 (the
prescale is baked into Q on the host).  exp then splits across engines by a
static per-pair schedule (PATTERN, 16 s-chunk pairs per 1024-q group):

  C: mask via PE matmul (lhsT = 128*I e5m2, mq in {-96,0} -> adds -12288 to
     masked scores); ScalarE ACT Exp(scale=ln2/128) reads PSUM -> E bf16.
  B: DVE TT-add mq in {0,-12288} f32 -> W in SBUF (batched [128,4096] over
     2 pairs); one ScalarE ACT Exp N=4096 -> E bf16.
  A: DVE TT-add mq in {16384,4096} -> int16, bitcast to bf16 = 2^(y/128+1)
     approx (Schraudolph); mean multiplier 1/ASCALE baked into that chunk's
     V slice on the host. Masked lanes land at 2^-95 ~= 0.  No ScalarE work.

PV: [O^T; den] += [V|1]^T @ E per chunk, f32 PSUM accumulation; acc drained
to SBUF (ScE qblk0 / DVE qblk1), DMA'd out as [65, q]; host divides by den
row and transposes.  No max-subtraction needed: scaled scores are ~N(0,1).
"""

import math
import sys

import numpy as np

_TRN_REPO = "/opt/trn_rl_repo"
if _TRN_REPO not in sys.path:
    sys.path.insert(0, _TRN_REPO)

import ml_dtypes  # noqa: E402

import concourse.bass as bass  # noqa: E402
import concourse.bacc as bacc  # noqa: E402
import concourse.tile as tile  # noqa: E402
from concourse import mybir  # noqa: E402

N_HEADS = 8
SEQ_Q = 4096
SEQ_S = 4096
D_HEAD = 64
V_HEAD = 64

SCH = 128  # s-chunk rows (psum partitions / PV contraction)
QBLK = 512  # one psum bank of f32
QGRP = 1024  # q columns per group
OCT = 8  # s-chunks per mask DMA

LOG2E = 1.4426950408889634
PRE = (1.0 / math.sqrt(D_HEAD)) * LOG2E * 128.0  # host Q prescale
LN2_128 = math.log(2.0) / 128.0  # ScalarE ACT scale

# Schraudolph A-path: j = int16(y + mq), mq in {16384 keep, 4096 masked};
# bitcast bf16 value = 2 * e^s * g(f), E[g] measured 1.0402 (round-nearest).
ASCALE_ROUND = 2.080431  # E[value/e^s] for round-to-nearest int16 conversion
ASCALE_TRUNC = 2.074800  # ... for truncation
ASCALE = ASCALE_ROUND  # set from probe/hw calibration

# Per-group schedule over 16 s-chunk pairs. B runs must have even length
# (each consecutive 2 B-pairs share one [128,4096] W batch); keep the last
# pairs B-free so their PV work is releasable before the group-end flush.
# Mix rationale: A (DVE 1-touch Schraudolph, 1.78% noise) capped ~44% by the
# accuracy gate; C (PE mask + ScE exp) and D (ScE exp + Gp mul) fill ScE/Gp
# under the PE roofline; Gp kept light (D=3) since its 2.1us ops add latency
# that can starve PV.  End with AA so the final group's drain chain is short.
PATTERN = "ABBDADADBBADADAA"
# PV release: max chunks released per pair, and readiness lag (pairs) per type
PV_RATE = 2
PV_LAG = {"A": 1, "B": 2, "C": 1, "D": 2}
PV_CARRY = 6  # max chunks left pending across a group boundary

_F32 = mybir.dt.float32
_BF16 = mybir.dt.bfloat16
_E5 = mybir.dt.float8e5
_I16 = mybir.dt.int16

_NP_BF16 = ml_dtypes.bfloat16
_NP_E5 = ml_dtypes.float8_e5m2

# mask-plane values per type: (keep, masked)
MQ_VALS = {"A": (16384.0, 4096.0), "B": (0.0, -12288.0), "C": (0.0, -96.0), "D": (1.0, 0.0)}


def _b_batches(pattern):
    """Group consecutive B pairs into batches of 2 (or 1 leftover)."""
    batches = {}  # pair_idx -> (batch_start_pair, size_pairs, slot)
    runs = []
    i = 0
    while i < len(pattern):
        if pattern[i] == "B":
            j = i
            while j < len(pattern) and pattern[j] == "B":
                j += 1
            runs.append((i, j))
            i = j
        else:
            i += 1
    for (i, j) in runs:
        p = i
        while p < j:
            size = 2 if p + 1 < j else 1
            for k in range(size):
                batches[p + k] = (p, size, k)
            p += size
    return batches


def build_nc(q_len=SEQ_Q, s_len=SEQ_S, d=D_HEAD, v=V_HEAD):
    """Single-core Bass graph (SPMD: same graph on every core)."""
    assert q_len % QGRP == 0 and s_len % (2 * SCH) == 0
    assert d == 64 and v == 64
    n_sc = s_len // SCH
    n_grp = q_len // QGRP
    nb = QGRP // QBLK  # 2
    n_pair = n_sc // 2
    assert n_pair == len(PATTERN)
    oct_sz = min(OCT, n_sc)
    n_oct = n_sc // oct_sz
    exp_fn = mybir.ActivationFunctionType.Exp
    bmap = _b_batches(PATTERN)

    # statics: one packed f32 tensor -> single logical input
    o_kt = q_len // 2  # qt bf16 cols / 2
    o_vp = o_kt + s_len // 2
    vpw = n_sc * (v + 1) // 2
    o_id = o_vp + vpw
    n_static = o_id + SCH // 2  # idq bf16 [128,128] = 64 f32 cols
    nc = bacc.Bacc("TRN2")
    statics = nc.dram_tensor("statics", (SCH, n_static), _F32, kind="ExternalInput")
    mt = nc.dram_tensor("mt", (n_grp, SCH, n_sc, QGRP), _E5, kind="ExternalInput")
    out_d = nc.dram_tensor("out", (v + 1, q_len), _F32, kind="ExternalOutput")

    with tile.TileContext(nc) as tc:
        with (
            tc.tile_pool(name="singles", bufs=1) as singles,
            tc.tile_pool(name="mpool", bufs=5) as mpool,
            tc.tile_pool(name="wpool", bufs=2) as wpool,
            tc.tile_pool(name="ebpool", bufs=2) as ebpool,
            tc.tile_pool(name="eapool", bufs=4) as eapool,
            tc.tile_pool(name="ecpool", bufs=4) as ecpool,
            tc.tile_pool(name="opool", bufs=2) as opool,
            tc.tile_pool(name="psS", bufs=3, space=bass.MemorySpace.PSUM) as psS,
            tc.tile_pool(name="psA", bufs=2, space=bass.MemorySpace.PSUM) as psA,
        ):
            # statics in first-use order so PE starts early; ALL streaming
            # loads ride the sync queue so DMA-ring FIFO order == need order.
            KT_HEAD = min(8, n_sc)
            kt_sb0 = singles.tile([SCH, KT_HEAD * 64], _F32)
            nc.sync.dma_start(
                out=kt_sb0[:, 0:128], in_=statics[:, o_kt : o_kt + 128]
            )
            qt_sb0 = singles.tile([SCH, QGRP // 2], _F32)
            nc.sync.dma_start(out=qt_sb0[:, 0 : QBLK // 2], in_=statics[:, 0 : QBLK // 2])
            misc_sb = singles.tile([SCH, n_static - o_vp], _F32)
            nc.sync.dma_start(
                out=misc_sb[:, o_id - o_vp :], in_=statics[:, o_id:]
            )
            kt_sb1 = None
            if n_sc > KT_HEAD:
                kt_sb1 = singles.tile([SCH, (n_sc - KT_HEAD) * 64], _F32)
            qt_sb1 = None
            if q_len > QGRP:
                qt_sb1 = singles.tile([SCH, (q_len - QGRP) // 2], _F32)

            def _late_loads():
                yield lambda: nc.sync.dma_start(
                    out=qt_sb0[:, QBLK // 2 : QGRP // 2],
                    in_=statics[:, QBLK // 2 : QGRP // 2],
                )
                yield lambda: nc.sync.dma_start(
                    out=kt_sb0[:, 128 : KT_HEAD * 64],
                    in_=statics[:, o_kt + 128 : o_kt + KT_HEAD * 64],
                )
                yield lambda: nc.sync.dma_start(
                    out=misc_sb[:, 0 : o_id - o_vp], in_=statics[:, o_vp : o_id]
                )
                if kt_sb1 is not None:
                    kw = (n_sc - KT_HEAD) * 64
                    for k in range(3):
                        s0, s1 = k * kw // 3, (k + 1) * kw // 3
                        yield lambda s0=s0, s1=s1: nc.sync.dma_start(
                            out=kt_sb1[:, s0:s1],
                            in_=statics[:, o_kt + KT_HEAD * 64 + s0 : o_kt + KT_HEAD * 64 + s1],
                        )
                if qt_sb1 is not None:
                    qw = (q_len - QGRP) // 2
                    for k in range(3):
                        s0, s1 = k * qw // 3, (k + 1) * qw // 3
                        yield lambda s0=s0, s1=s1: nc.sync.dma_start(
                            out=qt_sb1[:, s0:s1],
                            in_=statics[:, QGRP // 2 + s0 : QGRP // 2 + s1],
                        )

            late_loads = _late_loads()

            def drain_late_loads(k):
                for _ in range(k):
                    th = next(late_loads, None)
                    if th is not None:
                        th()

            def qt_slice(rows, q0_, q1_):
                if q0_ < QGRP:
                    return qt_sb0.bitcast(_BF16)[rows, q0_:q1_]
                return qt_sb1.bitcast(_BF16)[rows, q0_ - QGRP : q1_ - QGRP]

            def kt_slice(rows, c0_, c1_):
                if c0_ < KT_HEAD * SCH:
                    return kt_sb0.bitcast(_BF16)[rows, c0_:c1_]
                h = KT_HEAD * SCH
                return kt_sb1.bitcast(_BF16)[rows, c0_ - h : c1_ - h]

            # HAM warmup: fill the DMA-gated start window with junk matmuls.
            # memset on DVE so the first junk matmul isn't gated on a slow
            # GpSimd launch.
            warm_sb = singles.tile([64, 640], _BF16)
            nc.gpsimd.memset(warm_sb, 0.5)
            gwarm = singles.tile([64, 16], _BF16)
            nc.gpsimd.tensor_mul(gwarm, warm_sb[:, 0:16], warm_sb[:, 16:32])
            wps = psS.tile([SCH, QGRP], _F32, tag="S")
            for _wi in range(16):
                nc.tensor.matmul(
                    wps[:, 0:QBLK],
                    lhsT=warm_sb[:, 0:SCH],
                    rhs=warm_sb[:, SCH : SCH + QBLK],
                    start=True,
                    stop=True,
                    tile_position=(0, 0),
                )

            vp_sb = misc_sb[:, 0:vpw].bitcast(_BF16).rearrange(
                "p (c x) -> p c x", x=v + 1
            )
            idq_sb = misc_sb[:, o_id - o_vp : o_id - o_vp + SCH // 2].bitcast(_BF16)

            pend = []  # [(global_pair, type, E_ap, chunk, grp_ctx)] across groups

            def issue_pv(entry):
                _, _, e_ap, ci, ctx = entry
                first = ctx["n"] == 0
                last = ctx["n"] == n_sc - 1
                for qb in range(nb):
                    cs = slice(qb * QBLK, (qb + 1) * QBLK)
                    nc.tensor.matmul(
                        ctx["accs"][qb],
                        lhsT=vp_sb[:, ci, :],
                        rhs=e_ap[:, cs],
                        start=first,
                        stop=last,
                    )
                ctx["n"] += 1
                if last:
                    # drain acc -> sbuf (ScE qblk0, DVE qblk1), DMA out
                    nc.scalar.copy(ctx["out_sb"][:, 0:QBLK], ctx["accs"][0])
                    nc.vector.tensor_copy(
                        ctx["out_sb"][:, QBLK:QGRP], ctx["accs"][1]
                    )
                    nc.scalar.dma_start(
                        out=out_d[:, ctx["q0"] : ctx["q0"] + QGRP],
                        in_=ctx["out_sb"],
                    )

            def release_pv(gp, limit=PV_RATE):
                """Issue up to `limit` pending PV chunks whose E op was issued
                >= PV_LAG pairs ago (keeps PE fed, avoids convoys)."""
                n = 0
                i = 0
                while i < len(pend) and n < limit:
                    ip, ty_, _, _, _ = pend[i]
                    if gp - ip >= PV_LAG[ty_]:
                        issue_pv(pend.pop(i))
                        n += 1
                    else:
                        i += 1

            for g in range(n_grp):
                q0 = g * QGRP
                acc0 = psA.tile([v + 1, QBLK], _F32, tag="acc", bufs=2)
                acc1 = psA.tile([v + 1, QBLK], _F32, tag="acc", bufs=2)
                out_sb = opool.tile([v + 1, QGRP], _F32, tag="osb")
                ctx = {"accs": [acc0, acc1], "out_sb": out_sb, "q0": q0, "n": 0}

                wbatch = {}  # batch_start_pair -> (W_tile, E_tile)
                mq_tiles = {}

                for oc in range(n_oct):
                    mq = mpool.tile([SCH, oct_sz, QGRP], _E5, tag="mq")
                    nc.sync.dma_start(
                        out=mq,
                        in_=mt[g, :, oc * oct_sz : (oc + 1) * oct_sz, :],
                    )
                    if g == 0:
                        drain_late_loads(3)
                    for ci in range(oc * oct_sz, (oc + 1) * oct_sz):
                        mq_tiles[ci] = (mq, ci - oc * oct_sz)
                    for pp in range(oct_sz // 2):
                        p = oc * (oct_sz // 2) + pp
                        gp = g * n_pair + p
                        ty = PATTERN[p]
                        sa, sb_ = 2 * p, 2 * p + 1
                        # QK^T: row-packed pair (K=64 each, PE rows 0-63/64-127)
                        Sa = psS.tile([SCH, QGRP], _F32, tag="S")
                        Sb = psS.tile([SCH, QGRP], _F32, tag="S")
                        for qb in range(nb):
                            qs = slice(q0 + qb * QBLK, q0 + (qb + 1) * QBLK)
                            nc.tensor.matmul(
                                Sa[:, qb * QBLK : (qb + 1) * QBLK],
                                lhsT=kt_slice(slice(0, d), sa * SCH, (sa + 1) * SCH),
                                rhs=qt_slice(slice(0, d), qs.start, qs.stop),
                                start=True,
                                stop=ty != "C",
                                tile_position=(0, 0),
                            )
                        for qb in range(nb):
                            qs = slice(q0 + qb * QBLK, q0 + (qb + 1) * QBLK)
                            nc.tensor.matmul(
                                Sb[:, qb * QBLK : (qb + 1) * QBLK],
                                lhsT=kt_slice(slice(d, 2 * d), sb_ * SCH, (sb_ + 1) * SCH),
                                rhs=qt_slice(slice(d, 2 * d), qs.start, qs.stop),
                                start=True,
                                stop=ty != "C",
                                tile_position=(64, 0),
                            )
                        mqa_t, mqa_i = mq_tiles[sa]
                        mqb_t, mqb_i = mq_tiles[sb_]
                        mqa = mqa_t[:, mqa_i, :]
                        mqb = mqb_t[:, mqb_i, :]

                        if ty == "C":
                            # mask on PE: += (128*I)^T @ mq, then ScE exp
                            for qb in range(nb):
                                cs = slice(qb * QBLK, (qb + 1) * QBLK)
                                nc.tensor.matmul(
                                    Sa[:, cs], lhsT=idq_sb, rhs=mqa[:, cs],
                                    start=False, stop=True,
                                )
                                nc.tensor.matmul(
                                    Sb[:, cs], lhsT=idq_sb, rhs=mqb[:, cs],
                                    start=False, stop=True,
                                )
                            Ea = ecpool.tile([SCH, QGRP], _BF16, tag="eC", bufs=6)
                            Eb = ecpool.tile([SCH, QGRP], _BF16, tag="eC", bufs=6)
                            nc.scalar.activation(
                                out=Ea, in_=Sa, func=exp_fn, bias=0.0, scale=LN2_128
                            )
                            nc.scalar.activation(
                                out=Eb, in_=Sb, func=exp_fn, bias=0.0, scale=LN2_128
                            )
                            pend.append((gp, "C", Ea, sa, ctx))
                            pend.append((gp, "C", Eb, sb_, ctx))
                        elif ty == "D":
                            # ScE exp (no mask), GpSimd post-multiply by mq {1,0}
                            Ra = ecpool.tile([SCH, QGRP], _BF16, tag="eR", bufs=4)
                            Rb = ecpool.tile([SCH, QGRP], _BF16, tag="eR", bufs=4)
                            nc.scalar.activation(
                                out=Ra, in_=Sa, func=exp_fn, bias=0.0, scale=LN2_128
                            )
                            nc.scalar.activation(
                                out=Rb, in_=Sb, func=exp_fn, bias=0.0, scale=LN2_128
                            )
                            Ea = ecpool.tile([SCH, QGRP], _BF16, tag="eD", bufs=6)
                            Eb = ecpool.tile([SCH, QGRP], _BF16, tag="eD", bufs=6)
                            nc.gpsimd.tensor_mul(Ea, Ra, mqa)
                            nc.gpsimd.tensor_mul(Eb, Rb, mqb)
                            pend.append((gp, "D", Ea, sa, ctx))
                            pend.append((gp, "D", Eb, sb_, ctx))
                        elif ty == "A":
                            # fused mask+exp on DVE -> int16, bitcast bf16
                            Ja = eapool.tile([SCH, QGRP], _I16, tag="jA", bufs=6)
                            Jb = eapool.tile([SCH, QGRP], _I16, tag="jA", bufs=6)
                            nc.vector.tensor_add(Ja, Sa, mqa)
                            nc.vector.tensor_add(Jb, Sb, mqb)
                            pend.append((gp, "A", Ja.bitcast(_BF16), sa, ctx))
                            pend.append((gp, "A", Jb.bitcast(_BF16), sb_, ctx))
                        else:  # B: DVE mask-add -> W sbuf; batched ScE exp
                            bstart, bsize, slot = bmap[p]
                            if slot == 0:
                                W = wpool.tile(
                                    [SCH, bsize * 2 * QGRP], _F32, tag="W"
                                )
                                EB = ebpool.tile(
                                    [SCH, bsize * 2 * QGRP], _BF16, tag="eB"
                                )
                                wbatch[bstart] = (W, EB)
                            W, EB = wbatch[bstart]
                            wo = slot * 2 * QGRP
                            nc.vector.tensor_add(W[:, wo : wo + QGRP], Sa, mqa)
                            nc.vector.tensor_add(
                                W[:, wo + QGRP : wo + 2 * QGRP], Sb, mqb
                            )
                            if slot == bsize - 1:
                                nc.scalar.activation(
                                    out=EB, in_=W, func=exp_fn, bias=0.0,
                                    scale=LN2_128,
                                )
                                for k in range(bsize):
                                    pb = bstart + k
                                    pend.append(
                                        (gp, "B", EB[:, (2 * k) * QGRP : (2 * k + 1) * QGRP], 2 * pb, ctx)
                                    )
                                    pend.append(
                                        (gp, "B", EB[:, (2 * k + 1) * QGRP : (2 * k + 2) * QGRP], 2 * pb + 1, ctx)
                                    )
                        release_pv(gp)
                # trim backlog at group boundary (carry the rest)
                while len(pend) > PV_CARRY:
                    issue_pv(pend.pop(0))
            # final flush
            while pend:
                issue_pv(pend.pop(0))

    nc.compile()
    return nc


def prep_head(Qh, Kh, Vh, Mh):
    """Host-side layout prep for one head -> the core's input map."""
    q_len, d = Qh.shape
    s_len, v = Vh.shape
    n_sc = s_len // SCH
    n_grp = q_len // QGRP
    n_pair = n_sc // 2

    o_kt = q_len // 2
    o_vp = o_kt + s_len // 2
    vpw = n_sc * (v + 1) // 2
    o_id = o_vp + vpw
    n_static = o_id + SCH // 2
    statics = np.zeros((SCH, n_static), dtype=np.float32)
    qt = (np.asarray(Qh, np.float32).T * np.float32(PRE)).astype(_NP_BF16)
    qt2 = np.ascontiguousarray(np.concatenate([qt, qt], axis=0))  # [128, q]
    statics[:, 0 : q_len // 2] = qt2.view(np.float32)
    kt = np.asarray(Kh, np.float32).T.astype(_NP_BF16)  # [d, s]
    kt2 = np.ascontiguousarray(np.concatenate([kt, kt], axis=0))
    statics[:, o_kt : o_kt + s_len // 2] = kt2.view(np.float32)

    # [V|1] per chunk, with 1/ASCALE baked into A-chunks
    vpad = np.concatenate(
        [np.asarray(Vh, np.float32), np.ones((s_len, 1), np.float32)], axis=1
    ).reshape(n_sc, SCH, v + 1)
    ctype = np.array([PATTERN[c // 2] for c in range(n_sc)])
    vpad[ctype == "A"] *= np.float32(1.0 / ASCALE)
    vp = vpad.astype(_NP_BF16)  # [c, p, x]
    vp_p = np.ascontiguousarray(vp.transpose(1, 0, 2)).reshape(SCH, n_sc * (v + 1))
    statics[:, o_vp : o_vp + vpw] = vp_p.view(np.float32)
    idq = (128.0 * np.eye(SCH, dtype=np.float32)).astype(_NP_BF16)
    statics[:, o_id : o_id + SCH // 2] = idq.view(np.float32)

    # mask planes [s, q] with per-chunk-type values
    m = np.asarray(Mh, bool).T.reshape(n_sc, SCH, q_len)
    keep = np.array([MQ_VALS[t][0] for t in ctype], np.float32)[:, None, None]
    msk = np.array([MQ_VALS[t][1] for t in ctype], np.float32)[:, None, None]
    mv = np.where(m, keep, msk).astype(_NP_E5)  # [c, p, q]
    mt = np.ascontiguousarray(
        mv.reshape(n_sc, SCH, n_grp, QGRP).transpose(2, 1, 0, 3)
    )
    return {"statics": statics, "mt": mt}


_NC_CACHE = {}


def get_nc(q_len=SEQ_Q, s_len=SEQ_S, d=D_HEAD, v=V_HEAD):
    key = (q_len, s_len, d, v)
    if key not in _NC_CACHE:
        _NC_CACHE[key] = build_nc(*key)
    return _NC_CACHE[key]


def run_on_device(in_maps, nc=None, trace=False):
    from concourse.bass_utils import run_bass_kernel_spmd

    if nc is None:
        nc = get_nc()
    return run_bass_kernel_spmd(
        nc, in_maps, core_ids=list(range(len(in_maps))), trace=trace
    )


def finalize(raw):
    """[65, q] raw accumulator -> [q, v] normalized output."""
    return np.ascontiguousarray((raw[:V_HEAD] / raw[V_HEAD : V_HEAD + 1]).T)


def kernel(queries_nqd, keys_nsd, values_nsv, attention_mask_nqs):
    Q = np.asarray(queries_nqd, dtype=np.float32)
    K = np.asarray(keys_nsd, dtype=np.float32)
    V = np.asarray(values_nsv, dtype=np.float32)
    M = np.asarray(attention_mask_nqs, dtype=bool)
    n, q_len, d = Q.shape
    s_len, v = V.shape[1], V.shape[2]

    nc = get_nc(q_len, s_len, d, v)
    in_maps = [prep_head(Q[i], K[i], V[i], M[i]) for i in range(n)]
    res = run_on_device(in_maps, nc=nc)
    out = np.stack(
        [finalize(np.asarray(r["out"], dtype=np.float32)) for r in res.results],
        axis=0,
    )
    return np.ascontiguousarray(out)



# revision 30
# speedup vs baseline: 1.0900x; 1.0900x over previous
"""Trainium2 Bass kernel v3: masked dot-product attention, one head per core.

Per head: O = softmax(mask ? QK^T/sqrt(d) : -inf) @ V, all in "transposed"
[s, q] layout so PV needs no transpose.

Scores are computed in a log2*128 domain: PSUM y = S*scale*log2e*128 (the
prescale is baked into Q on the host).  exp then splits across engines by a
static per-pair schedule (PATTERN, 16 s-chunk pairs per 1024-q group):

  C: mask via PE matmul (lhsT = 128*I e5m2, mq in {-96,0} -> adds -12288 to
     masked scores); ScalarE ACT Exp(scale=ln2/128) reads PSUM -> E bf16.
  B: DVE TT-add mq in {0,-12288} f32 -> W in SBUF (batched [128,4096] over
     2 pairs); one ScalarE ACT Exp N=4096 -> E bf16.
  A: DVE TT-add mq in {16384,4096} -> int16, bitcast to bf16 = 2^(y/128+1)
     approx (Schraudolph); mean multiplier 1/ASCALE baked into that chunk's
     V slice on the host. Masked lanes land at 2^-95 ~= 0.  No ScalarE work.

PV: [O^T; den] += [V|1]^T @ E per chunk, f32 PSUM accumulation; acc drained
to SBUF (ScE qblk0 / DVE qblk1), DMA'd out as [65, q]; host divides by den
row and transposes.  No max-subtraction needed: scaled scores are ~N(0,1).
"""

import math
import sys

import numpy as np

_TRN_REPO = "/opt/trn_rl_repo"
if _TRN_REPO not in sys.path:
    sys.path.insert(0, _TRN_REPO)

import ml_dtypes  # noqa: E402

import concourse.bass as bass  # noqa: E402
import concourse.bacc as bacc  # noqa: E402
import concourse.tile as tile  # noqa: E402
from concourse import mybir  # noqa: E402

N_HEADS = 8
SEQ_Q = 4096
SEQ_S = 4096
D_HEAD = 64
V_HEAD = 64

SCH = 128  # s-chunk rows (psum partitions / PV contraction)
QBLK = 512  # one psum bank of f32
QGRP = 1024  # q columns per group
OCT = 8  # s-chunks per mask DMA

LOG2E = 1.4426950408889634
PRE = (1.0 / math.sqrt(D_HEAD)) * LOG2E * 128.0  # host Q prescale
LN2_128 = math.log(2.0) / 128.0  # ScalarE ACT scale

# Schraudolph A-path: j = int16(y + mq), mq in {16384 keep, 4096 masked};
# bitcast bf16 value = 2 * e^s * g(f), E[g] measured 1.0402 (round-nearest).
ASCALE_ROUND = 2.080431  # E[value/e^s] for round-to-nearest int16 conversion
ASCALE_TRUNC = 2.074800  # ... for truncation
ASCALE = ASCALE_ROUND  # set from probe/hw calibration

# Per-group schedule over 16 s-chunk pairs. B runs must have even length
# (each consecutive 2 B-pairs share one [128,4096] W batch); keep the last
# pairs B-free so their PV work is releasable before the group-end flush.
# Mix rationale: A (DVE 1-touch Schraudolph, 1.78% noise) capped ~44% by the
# accuracy gate; C (PE mask + ScE exp) and D (ScE exp + Gp mul) fill ScE/Gp
# under the PE roofline; Gp kept light (D=3) since its 2.1us ops add latency
# that can starve PV.  End with AA so the final group's drain chain is short.
PATTERN = "ACADACBBACADACAD"
# PV release: max chunks released per pair, and readiness lag (pairs) per type
PV_RATE = 2
PV_LAG = {"A": 1, "B": 2, "C": 1, "D": 2}
PV_CARRY = 6  # max chunks left pending across a group boundary

_F32 = mybir.dt.float32
_BF16 = mybir.dt.bfloat16
_E5 = mybir.dt.float8e5
_I16 = mybir.dt.int16

_NP_BF16 = ml_dtypes.bfloat16
_NP_E5 = ml_dtypes.float8_e5m2

# mask-plane values per type: (keep, masked)
MQ_VALS = {"A": (16384.0, 4096.0), "B": (0.0, -12288.0), "C": (0.0, -96.0), "D": (1.0, 0.0)}


def _b_batches(pattern):
    """Group consecutive B pairs into batches of 2 (or 1 leftover)."""
    batches = {}  # pair_idx -> (batch_start_pair, size_pairs, slot)
    runs = []
    i = 0
    while i < len(pattern):
        if pattern[i] == "B":
            j = i
            while j < len(pattern) and pattern[j] == "B":
                j += 1
            runs.append((i, j))
            i = j
        else:
            i += 1
    for (i, j) in runs:
        p = i
        while p < j:
            size = 2 if p + 1 < j else 1
            for k in range(size):
                batches[p + k] = (p, size, k)
            p += size
    return batches


def build_nc(q_len=SEQ_Q, s_len=SEQ_S, d=D_HEAD, v=V_HEAD):
    """Single-core Bass graph (SPMD: same graph on every core)."""
    assert q_len % QGRP == 0 and s_len % (2 * SCH) == 0
    assert d == 64 and v == 64
    n_sc = s_len // SCH
    n_grp = q_len // QGRP
    nb = QGRP // QBLK  # 2
    n_pair = n_sc // 2
    assert n_pair == len(PATTERN)
    oct_sz = min(OCT, n_sc)
    n_oct = n_sc // oct_sz
    exp_fn = mybir.ActivationFunctionType.Exp
    bmap = _b_batches(PATTERN)

    # statics: one packed f32 tensor -> single logical input
    o_kt = q_len // 2  # qt bf16 cols / 2
    o_vp = o_kt + s_len // 2
    vpw = n_sc * (v + 1) // 2
    o_id = o_vp + vpw
    n_static = o_id + SCH // 2  # idq bf16 [128,128] = 64 f32 cols
    nc = bacc.Bacc("TRN2")
    statics = nc.dram_tensor("statics", (SCH, n_static), _F32, kind="ExternalInput")
    mt = nc.dram_tensor("mt", (n_grp, SCH, n_sc, QGRP), _E5, kind="ExternalInput")
    out_d = nc.dram_tensor("out", (v + 1, q_len), _F32, kind="ExternalOutput")

    with tile.TileContext(nc) as tc:
        with (
            tc.tile_pool(name="singles", bufs=1) as singles,
            tc.tile_pool(name="mpool", bufs=8) as mpool,
            tc.tile_pool(name="wpool", bufs=2) as wpool,
            tc.tile_pool(name="ebpool", bufs=2) as ebpool,
            tc.tile_pool(name="eapool", bufs=4) as eapool,
            tc.tile_pool(name="ecpool", bufs=4) as ecpool,
            tc.tile_pool(name="opool", bufs=2) as opool,
            tc.tile_pool(name="psS", bufs=3, space=bass.MemorySpace.PSUM) as psS,
            tc.tile_pool(name="psA", bufs=2, space=bass.MemorySpace.PSUM) as psA,
        ):
            # statics in first-use order so PE starts early; ALL streaming
            # loads ride the sync queue so DMA-ring FIFO order == need order.
            KT_HEAD = min(8, n_sc)
            kt_sb0 = singles.tile([SCH, KT_HEAD * 64], _F32)
            nc.sync.dma_start(
                out=kt_sb0[:, 0:128], in_=statics[:, o_kt : o_kt + 128]
            )
            qt_sb0 = singles.tile([SCH, QGRP // 2], _F32)
            nc.sync.dma_start(out=qt_sb0[:, 0 : QBLK // 2], in_=statics[:, 0 : QBLK // 2])
            misc_sb = singles.tile([SCH, n_static - o_vp], _F32)
            nc.sync.dma_start(
                out=misc_sb[:, o_id - o_vp :], in_=statics[:, o_id:]
            )
            kt_sb1 = None
            if n_sc > KT_HEAD:
                kt_sb1 = singles.tile([SCH, (n_sc - KT_HEAD) * 64], _F32)
            qt_sb1 = None
            if q_len > QGRP:
                qt_sb1 = singles.tile([SCH, (q_len - QGRP) // 2], _F32)

            def _late_loads():
                yield lambda: nc.sync.dma_start(
                    out=qt_sb0[:, QBLK // 2 : QGRP // 2],
                    in_=statics[:, QBLK // 2 : QGRP // 2],
                )
                yield lambda: nc.sync.dma_start(
                    out=kt_sb0[:, 128 : KT_HEAD * 64],
                    in_=statics[:, o_kt + 128 : o_kt + KT_HEAD * 64],
                )
                yield lambda: nc.sync.dma_start(
                    out=misc_sb[:, 0 : o_id - o_vp], in_=statics[:, o_vp : o_id]
                )
                if kt_sb1 is not None:
                    kw = (n_sc - KT_HEAD) * 64
                    for k in range(3):
                        s0, s1 = k * kw // 3, (k + 1) * kw // 3
                        yield lambda s0=s0, s1=s1: nc.sync.dma_start(
                            out=kt_sb1[:, s0:s1],
                            in_=statics[:, o_kt + KT_HEAD * 64 + s0 : o_kt + KT_HEAD * 64 + s1],
                        )
                if qt_sb1 is not None:
                    qw = (q_len - QGRP) // 2
                    for k in range(3):
                        s0, s1 = k * qw // 3, (k + 1) * qw // 3
                        yield lambda s0=s0, s1=s1: nc.sync.dma_start(
                            out=qt_sb1[:, s0:s1],
                            in_=statics[:, QGRP // 2 + s0 : QGRP // 2 + s1],
                        )

            late_loads = _late_loads()

            def drain_late_loads(k):
                for _ in range(k):
                    th = next(late_loads, None)
                    if th is not None:
                        th()

            def qt_slice(rows, q0_, q1_):
                if q0_ < QGRP:
                    return qt_sb0.bitcast(_BF16)[rows, q0_:q1_]
                return qt_sb1.bitcast(_BF16)[rows, q0_ - QGRP : q1_ - QGRP]

            def kt_slice(rows, c0_, c1_):
                if c0_ < KT_HEAD * SCH:
                    return kt_sb0.bitcast(_BF16)[rows, c0_:c1_]
                h = KT_HEAD * SCH
                return kt_sb1.bitcast(_BF16)[rows, c0_ - h : c1_ - h]

            # HAM warmup: fill the DMA-gated start window with junk matmuls.
            # memset on DVE so the first junk matmul isn't gated on a slow
            # GpSimd launch.
            warm_sb = singles.tile([64, 640], _BF16)
            nc.gpsimd.memset(warm_sb, 0.5)
            gwarm = singles.tile([64, 16], _BF16)
            nc.gpsimd.tensor_mul(gwarm, warm_sb[:, 0:16], warm_sb[:, 16:32])
            wps = psS.tile([SCH, QGRP], _F32, tag="S")
            for _wi in range(16):
                nc.tensor.matmul(
                    wps[:, 0:QBLK],
                    lhsT=warm_sb[:, 0:SCH],
                    rhs=warm_sb[:, SCH : SCH + QBLK],
                    start=True,
                    stop=True,
                    tile_position=(0, 0),
                )

            vp_sb = misc_sb[:, 0:vpw].bitcast(_BF16).rearrange(
                "p (c x) -> p c x", x=v + 1
            )
            idq_sb = misc_sb[:, o_id - o_vp : o_id - o_vp + SCH // 2].bitcast(_BF16)

            pend = []  # [(global_pair, type, E_ap, chunk, grp_ctx)] across groups

            def issue_pv(entry):
                _, _, e_ap, ci, ctx = entry
                first = ctx["n"] == 0
                last = ctx["n"] == n_sc - 1
                for qb in range(nb):
                    cs = slice(qb * QBLK, (qb + 1) * QBLK)
                    nc.tensor.matmul(
                        ctx["accs"][qb],
                        lhsT=vp_sb[:, ci, :],
                        rhs=e_ap[:, cs],
                        start=first,
                        stop=last,
                    )
                ctx["n"] += 1
                if last:
                    # drain acc -> sbuf (ScE qblk0, DVE qblk1), DMA out
                    nc.scalar.copy(ctx["out_sb"][:, 0:QBLK], ctx["accs"][0])
                    nc.vector.tensor_copy(
                        ctx["out_sb"][:, QBLK:QGRP], ctx["accs"][1]
                    )
                    nc.scalar.dma_start(
                        out=out_d[:, ctx["q0"] : ctx["q0"] + QGRP],
                        in_=ctx["out_sb"],
                    )

            def release_pv(gp, limit=PV_RATE):
                """Issue up to `limit` pending PV chunks whose E op was issued
                >= PV_LAG pairs ago (keeps PE fed, avoids convoys)."""
                n = 0
                i = 0
                while i < len(pend) and n < limit:
                    ip, ty_, _, _, _ = pend[i]
                    if gp - ip >= PV_LAG[ty_]:
                        issue_pv(pend.pop(i))
                        n += 1
                    else:
                        i += 1

            for g in range(n_grp):
                q0 = g * QGRP
                acc0 = psA.tile([v + 1, QBLK], _F32, tag="acc", bufs=2)
                acc1 = psA.tile([v + 1, QBLK], _F32, tag="acc", bufs=2)
                out_sb = opool.tile([v + 1, QGRP], _F32, tag="osb")
                ctx = {"accs": [acc0, acc1], "out_sb": out_sb, "q0": q0, "n": 0}

                wbatch = {}  # batch_start_pair -> (W_tile, E_tile)
                mq_tiles = {}

                for oc in range(n_oct):
                    mq = mpool.tile([SCH, oct_sz, QGRP], _E5, tag="mq")
                    nc.sync.dma_start(
                        out=mq,
                        in_=mt[g, :, oc * oct_sz : (oc + 1) * oct_sz, :],
                    )
                    if g == 0:
                        drain_late_loads(3)
                    for ci in range(oc * oct_sz, (oc + 1) * oct_sz):
                        mq_tiles[ci] = (mq, ci - oc * oct_sz)
                    for pp in range(oct_sz // 2):
                        p = oc * (oct_sz // 2) + pp
                        gp = g * n_pair + p
                        ty = PATTERN[p]
                        sa, sb_ = 2 * p, 2 * p + 1
                        # QK^T: row-packed pair (K=64 each, PE rows 0-63/64-127)
                        Sa = psS.tile([SCH, QGRP], _F32, tag="S")
                        Sb = psS.tile([SCH, QGRP], _F32, tag="S")
                        for qb in range(nb):
                            qs = slice(q0 + qb * QBLK, q0 + (qb + 1) * QBLK)
                            nc.tensor.matmul(
                                Sa[:, qb * QBLK : (qb + 1) * QBLK],
                                lhsT=kt_slice(slice(0, d), sa * SCH, (sa + 1) * SCH),
                                rhs=qt_slice(slice(0, d), qs.start, qs.stop),
                                start=True,
                                stop=ty != "C",
                                tile_position=(0, 0),
                            )
                        for qb in range(nb):
                            qs = slice(q0 + qb * QBLK, q0 + (qb + 1) * QBLK)
                            nc.tensor.matmul(
                                Sb[:, qb * QBLK : (qb + 1) * QBLK],
                                lhsT=kt_slice(slice(d, 2 * d), sb_ * SCH, (sb_ + 1) * SCH),
                                rhs=qt_slice(slice(d, 2 * d), qs.start, qs.stop),
                                start=True,
                                stop=ty != "C",
                                tile_position=(64, 0),
                            )
                        mqa_t, mqa_i = mq_tiles[sa]
                        mqb_t, mqb_i = mq_tiles[sb_]
                        mqa = mqa_t[:, mqa_i, :]
                        mqb = mqb_t[:, mqb_i, :]

                        if ty == "C":
                            # mask on PE: += (128*I)^T @ mq, then ScE exp
                            for qb in range(nb):
                                cs = slice(qb * QBLK, (qb + 1) * QBLK)
                                nc.tensor.matmul(
                                    Sa[:, cs], lhsT=idq_sb, rhs=mqa[:, cs],
                                    start=False, stop=True,
                                )
                                nc.tensor.matmul(
                                    Sb[:, cs], lhsT=idq_sb, rhs=mqb[:, cs],
                                    start=False, stop=True,
                                )
                            Ea = ecpool.tile([SCH, QGRP], _BF16, tag="eC", bufs=6)
                            Eb = ecpool.tile([SCH, QGRP], _BF16, tag="eC", bufs=6)
                            nc.scalar.activation(
                                out=Ea, in_=Sa, func=exp_fn, bias=0.0, scale=LN2_128
                            )
                            nc.scalar.activation(
                                out=Eb, in_=Sb, func=exp_fn, bias=0.0, scale=LN2_128
                            )
                            pend.append((gp, "C", Ea, sa, ctx))
                            pend.append((gp, "C", Eb, sb_, ctx))
                        elif ty == "D":
                            # ScE exp (no mask), GpSimd post-multiply by mq {1,0}
                            Ra = ecpool.tile([SCH, QGRP], _BF16, tag="eR", bufs=4)
                            Rb = ecpool.tile([SCH, QGRP], _BF16, tag="eR", bufs=4)
                            nc.scalar.activation(
                                out=Ra, in_=Sa, func=exp_fn, bias=0.0, scale=LN2_128
                            )
                            nc.scalar.activation(
                                out=Rb, in_=Sb, func=exp_fn, bias=0.0, scale=LN2_128
                            )
                            Ea = ecpool.tile([SCH, QGRP], _BF16, tag="eD", bufs=6)
                            Eb = ecpool.tile([SCH, QGRP], _BF16, tag="eD", bufs=6)
                            nc.gpsimd.tensor_mul(Ea, Ra, mqa)
                            nc.gpsimd.tensor_mul(Eb, Rb, mqb)
                            pend.append((gp, "D", Ea, sa, ctx))
                            pend.append((gp, "D", Eb, sb_, ctx))
                        elif ty == "A":
                            # fused mask+exp on DVE -> int16, bitcast bf16
                            Ja = eapool.tile([SCH, QGRP], _I16, tag="jA", bufs=6)
                            Jb = eapool.tile([SCH, QGRP], _I16, tag="jA", bufs=6)
                            nc.vector.tensor_add(Ja, Sa, mqa)
                            nc.vector.tensor_add(Jb, Sb, mqb)
                            pend.append((gp, "A", Ja.bitcast(_BF16), sa, ctx))
                            pend.append((gp, "A", Jb.bitcast(_BF16), sb_, ctx))
                        else:  # B: DVE mask-add -> W sbuf; batched ScE exp
                            bstart, bsize, slot = bmap[p]
                            if slot == 0:
                                W = wpool.tile(
                                    [SCH, bsize * 2 * QGRP], _F32, tag="W"
                                )
                                EB = ebpool.tile(
                                    [SCH, bsize * 2 * QGRP], _BF16, tag="eB"
                                )
                                wbatch[bstart] = (W, EB)
                            W, EB = wbatch[bstart]
                            wo = slot * 2 * QGRP
                            nc.vector.tensor_add(W[:, wo : wo + QGRP], Sa, mqa)
                            nc.vector.tensor_add(
                                W[:, wo + QGRP : wo + 2 * QGRP], Sb, mqb
                            )
                            if slot == bsize - 1:
                                nc.scalar.activation(
                                    out=EB, in_=W, func=exp_fn, bias=0.0,
                                    scale=LN2_128,
                                )
                                for k in range(bsize):
                                    pb = bstart + k
                                    pend.append(
                                        (gp, "B", EB[:, (2 * k) * QGRP : (2 * k + 1) * QGRP], 2 * pb, ctx)
                                    )
                                    pend.append(
                                        (gp, "B", EB[:, (2 * k + 1) * QGRP : (2 * k + 2) * QGRP], 2 * pb + 1, ctx)
                                    )
                        release_pv(gp)
                # trim backlog at group boundary (carry the rest)
                while len(pend) > PV_CARRY:
                    issue_pv(pend.pop(0))
            # final flush
            while pend:
                issue_pv(pend.pop(0))

    nc.compile()
    return nc


def prep_head(Qh, Kh, Vh, Mh):
    """Host-side layout prep for one head -> the core's input map."""
    q_len, d = Qh.shape
    s_len, v = Vh.shape
    n_sc = s_len // SCH
    n_grp = q_len // QGRP
    n_pair = n_sc // 2

    o_kt = q_len // 2
    o_vp = o_kt + s_len // 2
    vpw = n_sc * (v + 1) // 2
    o_id = o_vp + vpw
    n_static = o_id + SCH // 2
    statics = np.zeros((SCH, n_static), dtype=np.float32)
    qt = (np.asarray(Qh, np.float32).T * np.float32(PRE)).astype(_NP_BF16)
    qt2 = np.ascontiguousarray(np.concatenate([qt, qt], axis=0))  # [128, q]
    statics[:, 0 : q_len // 2] = qt2.view(np.float32)
    kt = np.asarray(Kh, np.float32).T.astype(_NP_BF16)  # [d, s]
    kt2 = np.ascontiguousarray(np.concatenate([kt, kt], axis=0))
    statics[:, o_kt : o_kt + s_len // 2] = kt2.view(np.float32)

    # [V|1] per chunk, with 1/ASCALE baked into A-chunks
    vpad = np.concatenate(
        [np.asarray(Vh, np.float32), np.ones((s_len, 1), np.float32)], axis=1
    ).reshape(n_sc, SCH, v + 1)
    ctype = np.array([PATTERN[c // 2] for c in range(n_sc)])
    vpad[ctype == "A"] *= np.float32(1.0 / ASCALE)
    vp = vpad.astype(_NP_BF16)  # [c, p, x]
    vp_p = np.ascontiguousarray(vp.transpose(1, 0, 2)).reshape(SCH, n_sc * (v + 1))
    statics[:, o_vp : o_vp + vpw] = vp_p.view(np.float32)
    idq = (128.0 * np.eye(SCH, dtype=np.float32)).astype(_NP_BF16)
    statics[:, o_id : o_id + SCH // 2] = idq.view(np.float32)

    # mask planes [s, q] with per-chunk-type values
    m = np.asarray(Mh, bool).T.reshape(n_sc, SCH, q_len)
    keep = np.array([MQ_VALS[t][0] for t in ctype], np.float32)[:, None, None]
    msk = np.array([MQ_VALS[t][1] for t in ctype], np.float32)[:, None, None]
    mv = np.where(m, keep, msk).astype(_NP_E5)  # [c, p, q]
    mt = np.ascontiguousarray(
        mv.reshape(n_sc, SCH, n_grp, QGRP).transpose(2, 1, 0, 3)
    )
    return {"statics": statics, "mt": mt}


_NC_CACHE = {}


def get_nc(q_len=SEQ_Q, s_len=SEQ_S, d=D_HEAD, v=V_HEAD):
    key = (q_len, s_len, d, v)
    if key not in _NC_CACHE:
        _NC_CACHE[key] = build_nc(*key)
    return _NC_CACHE[key]


def run_on_device(in_maps, nc=None, trace=False):
    from concourse.bass_utils import run_bass_kernel_spmd

    if nc is None:
        nc = get_nc()
    return run_bass_kernel_spmd(
        nc, in_maps, core_ids=list(range(len(in_maps))), trace=trace
    )


def finalize(raw):
    """[65, q] raw accumulator -> [q, v] normalized output."""
    return np.ascontiguousarray((raw[:V_HEAD] / raw[V_HEAD : V_HEAD + 1]).T)


def kernel(queries_nqd, keys_nsd, values_nsv, attention_mask_nqs):
    Q = np.asarray(queries_nqd, dtype=np.float32)
    K = np.asarray(keys_nsd, dtype=np.float32)
    V = np.asarray(values_nsv, dtype=np.float32)
    M = np.asarray(attention_mask_nqs, dtype=bool)
    n, q_len, d = Q.shape
    s_len, v = V.shape[1], V.shape[2]

    nc = get_nc(q_len, s_len, d, v)
    in_maps = [prep_head(Q[i], K[i], V[i], M[i]) for i in range(n)]
    res = run_on_device(in_maps, nc=nc)
    out = np.stack(
        [finalize(np.asarray(r["out"], dtype=np.float32)) for r in res.results],
        axis=0,
    )
    return np.ascontiguousarray(out)

